# revision 1
# baseline (speedup 1.0000x reference)
"""Trainium2 Bass kernel for nn_DiscriminationLoss (segment_reduce).

Math: the reference loss reduces to, per image b:
  S[b,k,c]    = sum of pred[b,c] over pixels with label k   (k=1..16 needed)
  counts[b,k] = histogram of labels                          (k=0..16)
  Kb          = max label present (derived from counts on host)
followed by a tiny scalar epilogue:
  N = ||S||_2 over c, N[0]=0; f = log(relu(3-N)^2+1)
  sum_g = counts . f     (replaces the per-pixel gather in the reference)
  own/other/scale pair-combination and a final scalar sum.

Device work per core (2 images): for each (k,c) one fused DVE
scalar_tensor_tensor stream computes (labels == k) * pred_c and row-sums it
into an accumulator column; histogram via tensor_scalar(is_equal) with
accum_out; per-partition partials [128, 81] are DMA'd out and the epilogue
(tiny: 16 x 81 numbers) runs on host.

Toolchain constraints worked around here:
- walrus rejects sem waits riding on STT/TS compute and on HWDGE direct2d
  DMAs ("Too many sync wait commands"), so each input DMA's completion wait
  is absorbed by one tiny tensor_copy (which can carry waits) and all
  later consumers are same-engine ordered.
- the kernel-tail drain has limited wait slots, so inputs are loaded with
  exactly two big DMAs (a single InstDMACopy already fans out across all 16
  SDMA engines, so this costs no bandwidth) and the output leaves via one
  SWDGE (gpsimd) DMA.

Inputs are pre-converted to bf16 on host: labels 0..16 are exact in bf16;
pred quantization (~0.4% per element) is far below the relu(3-||S||)=0
margin (||S|| ~ 300 for every populated segment), and halves DMA bytes.

Sharding: data-parallel over batch, 2 images per core, no collectives.
"""

import json

import numpy as np
import ml_dtypes

import concourse.bass as bass
import concourse.mybir as mybir
import concourse.tile as tile
import concourse.bass2jax as _b2j
from concourse.bass_utils import run_bass_kernel_spmd


def _split_multiwait_bir(bir_json: bytes) -> bytes:
    """walrus in this container rejects instructions carrying more than one
    sync wait. Tile's kernel-tail drain aggregates one wait per DMA/engine
    sem lane onto a single SP Drain, so split any multi-wait instruction
    into single-wait predecessors on the same engine."""
    d = json.loads(bir_json)
    changed = False
    for fn in d.get("functions", []):
        for bb in fn.get("blocks", []):
            insts = bb.get("instructions", [])
            out = []
            for ins in insts:
                si = ins.get("sync_info") or {}
                waits = si.get("on_wait") or []
                if len(waits) > 1:
                    changed = True
                    for wi, w in enumerate(waits[:-1]):
                        out.append(
                            {
                                "debug": ins.get("debug"),
                                "engine": ins["engine"],
                                "ins": [],
                                "is_reset_sema": False,
                                "name": f"{ins['name']}_w{wi}",
                                "opcode": "Drain",
                                "outs": [],
                                "sync_info": {"on_update": [], "on_wait": [w]},
                            }
                        )
                    si["on_wait"] = [waits[-1]]
                out.append(ins)
            bb["instructions"] = out
    if not changed:
        return bir_json
    return json.dumps(d).encode()


_ORIG_COMPILE_BIR = _b2j.compile_bir_kernel


def _compile_bir_splitting_waits(bir_json, tmpdir, neff_name="file.neff"):
    return _ORIG_COMPILE_BIR(_split_multiwait_bir(bir_json), tmpdir, neff_name=neff_name)


_b2j.compile_bir_kernel = _compile_bir_splitting_waits

B, C, H, W = 16, 4, 640, 640
HW = H * W                 # 409600
P = 128
FD = HW // P               # 3200
N_CORES = 8
IPC = B // N_CORES         # images per core
KMAX = 16
K1 = KMAX + 1
SIGMA_DIS = 3.0
F0 = float(np.log(SIGMA_DIS**2 + 1.0))
NS = KMAX * C              # 64 segment-sum slots (k=1..16)
NACC = NS + K1             # + 17 histogram slots = 81

# test.py can set RUN_KWARGS["trace"] = True and read LAST_RESULT for profiling
RUN_KWARGS = {}
LAST_RESULT = None
_NC_CACHE = []

BF16 = mybir.dt.bfloat16
F32 = mybir.dt.float32


PRED_COLS = IPC * C * FD      # 25600
LAB_COLS = IPC * FD           # 6400
DATA_COLS = PRED_COLS + LAB_COLS
MCHUNK = 800                  # mask-chunk columns (pipeline DVE vs PE)
NCHUNK = FD // MCHUNK
# out layout: per-image [NACC] DVE region (S zeros + chunk-0 counts),
# per-image [KMAX] PE region (rows 0..C-1), then per-image extra count
# partials for mask-chunks 1.. (K1 cols each)
OUT_COLS = IPC * NACC + IPC * KMAX + IPC * (NCHUNK - 1) * K1


def _build_nc():
    """Segment sums on PE (per pixel-chunk matmul: pred [128,4] stationary,
    16 mask columns moving, f32 PSUM accumulation over 3200 chunks/image);
    masks + histogram on DVE (tensor_scalar, single-src perf modes). The two
    engines have separate SBUF ports and run concurrently."""
    nc = bass.Bass("TRN2", target_bir_lowering=False, debug=False)
    data = nc.dram_tensor("data", [P, DATA_COLS], BF16, kind="ExternalInput")
    out = nc.dram_tensor("out", [P, OUT_COLS], F32, kind="ExternalOutput")

    with tile.TileContext(nc) as tc:
        with tc.tile_pool(name="pool", bufs=1) as pool, \
             tc.tile_pool(name="ps", bufs=2, space="PSUM") as pspool:
            data_sb = pool.tile([P, DATA_COLS], BF16, name="data_sb")
            # split the load so compute starts as slices land: labels (small,
            # first) unblock DVE mask-building ~5us in; per-image pred
            # unblocks PE before the full 8.2MB is resident
            CFD = C * FD
            FH = FD // 2
            nc.sync.dma_start(data_sb[:, PRED_COLS:DATA_COLS],
                              data[:, PRED_COLS:DATA_COLS])
            # per-image pred in two column-half strided DMAs (3D AP spans all
            # 4 channel slabs), so PE starts once the first half is resident
            halves = []
            for i in range(IPC):
                sb_i = data_sb[:, i * CFD : (i + 1) * CFD].rearrange(
                    "p (c f) -> p c f", c=C)
                dr_i = data[:, i * CFD : (i + 1) * CFD].rearrange(
                    "p (c f) -> p c f", c=C)
                for h in range(2):
                    nc.sync.dma_start(sb_i[:, :, h * FH : (h + 1) * FH],
                                      dr_i[:, :, h * FH : (h + 1) * FH])
                    halves.append((i, h))
            # tiny DVE copies absorb each DMA-completion wait; later DVE/PE
            # consumers then order off the DVE sem (single wait each)
            dummy = pool.tile([P, 16], BF16, name="dummy")
            nc.vector.tensor_copy(dummy[:, 0:2], data_sb[:, PRED_COLS : PRED_COLS + 2])
            for n, (i, h) in enumerate(halves):
                lo = i * CFD + h * FH
                nc.vector.tensor_copy(dummy[:, 2 + 2 * n : 4 + 2 * n],
                                      data_sb[:, lo : lo + 2])

            acc = pool.tile([P, OUT_COLS], F32, name="acc")
            # PE region only gets rows 0..C-1 written; zero the rest once
            nc.vector.memset(acc[:, IPC * NACC : OUT_COLS], 0.0)
            NCH = FD // MCHUNK
            for i in range(IPC):
                lab = data_sb[:, PRED_COLS + i * FD : PRED_COLS + (i + 1) * FD]
                scratch = pool.tile([P, MCHUNK], BF16, name=f"scratch_{i}")
                # DVE S region unused -> zero so the host addition is valid
                nc.vector.memset(acc[:, i * NACC : i * NACC + NS], 0.0)
                pred_i = data_sb[:, i * C * FD : (i + 1) * C * FD].rearrange(
                    "p (c f) -> p c f", c=C
                )
                ps = pspool.tile([C, KMAX], F32, name=f"ps_{i}")
                # per-chunk count partials; summed with the host partition sum
                cnt = acc[:, i * NACC + NS : i * NACC + NS + K1]
                c2o = IPC * NACC + IPC * KMAX + i * (NCHUNK - 1) * K1
                cnt2 = acc[:, c2o : c2o + (NCHUNK - 1) * K1]
                for j in range(NCH):
                    lo = j * MCHUNK
                    # column-chunked k-major mask slab, double-buffered so
                    # mask building (DVE) pipelines against PE consumption
                    mk = pool.tile([P, KMAX * MCHUNK], BF16,
                                   name=f"mk_{i}_{j}", tag="mk", bufs=2)
                    for k in range(1, K1):
                        # fused: mask tile for PE + histogram row-count
                        nc.vector.tensor_scalar(
                            out=mk[:, (k - 1) * MCHUNK : k * MCHUNK],
                            in0=lab[:, lo : lo + MCHUNK],
                            scalar1=float(k),
                            scalar2=None,
                            op0=mybir.AluOpType.is_equal,
                            op1=mybir.AluOpType.add,
                            accum_out=cnt[:, k : k + 1] if j == 0 else
                                      cnt2[:, (j - 1) * K1 + k : (j - 1) * K1 + k + 1],
                        )
                    # k=0 count (mask itself not needed by PE)
                    nc.vector.tensor_scalar(
                        out=scratch[:],
                        in0=lab[:, lo : lo + MCHUNK],
                        scalar1=0.0,
                        scalar2=None,
                        op0=mybir.AluOpType.is_equal,
                        op1=mybir.AluOpType.add,
                        accum_out=cnt[:, 0:1] if j == 0 else
                                  cnt2[:, (j - 1) * K1 : (j - 1) * K1 + 1],
                    )
                    mk_r = mk[:].rearrange("p (k f) -> p k f", k=KMAX)
                    for t in range(MCHUNK):
                        nc.tensor.matmul(
                            ps[:],
                            pred_i[:, :, lo + t],
                            mk_r[:, :, t],
                            start=(j == 0 and t == 0),
                            stop=(j == NCH - 1 and t == MCHUNK - 1),
                        )
                # drain psum [C, KMAX] into the PE region (rows 0..C-1)
                po = IPC * NACC + i * KMAX
                nc.vector.tensor_copy(acc[0:C, po : po + KMAX], ps[:])
            # consolidate per-column accum deps into one DVE copy so the
            # single out DMA carries one sem wait
            acc_out = pool.tile([P, OUT_COLS], F32, name="acc_out")
            nc.vector.tensor_copy(acc_out[:], acc[:])
            nc.gpsimd.dma_start(out[:], acc_out[:])
    return nc


def _get_nc():
    if not _NC_CACHE:
        _NC_CACHE.append(_build_nc())
    return _NC_CACHE[0]


def _to_bf16(x: np.ndarray) -> np.ndarray:
    # round-to-nearest-even f32 -> bf16 via integer trick (fast numpy path)
    u = x.view(np.uint32)
    rounded = (u + 0x7FFF + ((u >> 16) & 1)) >> 16
    return rounded.astype(np.uint16).view(ml_dtypes.bfloat16)


def make_in_maps(pred_similarities, kernel_labels):
    pred = np.ascontiguousarray(pred_similarities, dtype=np.float32).reshape(
        N_CORES, IPC, C, P, FD
    )
    labs = np.ascontiguousarray(kernel_labels, dtype=np.int32)
    pred_bf = _to_bf16(pred)                       # [N_CORES, IPC, C, P, FD]
    labs_bf = labs.astype(np.float32).reshape(N_CORES, IPC, P, FD)
    labs_bf = _to_bf16(labs_bf)                    # exact for 0..16
    in_maps = []
    for i in range(N_CORES):
        # -> [P, IPC*C*FD] / [P, IPC*FD] with (image, channel) column-major,
        # packed into a single [P, DATA_COLS] tensor
        p = pred_bf[i].transpose(2, 0, 1, 3).reshape(P, IPC * C * FD)
        l = labs_bf[i].transpose(1, 0, 2).reshape(P, IPC * FD)
        in_maps.append({"data": np.ascontiguousarray(np.concatenate([p, l], axis=1))})
    return in_maps


def kernel(pred_similarities, kernel_labels):
    global LAST_RESULT
    nc = _get_nc()
    in_maps = make_in_maps(pred_similarities, kernel_labels)
    res = run_bass_kernel_spmd(nc, in_maps, core_ids=list(range(N_CORES)), **RUN_KWARGS)
    LAST_RESULT = res
    outs = [res.results[c]["out"] for c in range(N_CORES)]
    return epilogue(outs)


def epilogue(outs):
    S = np.zeros((B, K1, C), np.float64)
    counts = np.zeros((B, K1), np.float64)
    for core in range(N_CORES):
        o = np.asarray(outs[core]).astype(np.float64)  # [P, OUT_COLS]
        for i in range(IPC):
            b = core * IPC + i
            red = o[:, i * NACC : (i + 1) * NACC].sum(axis=0)  # partition partials
            S[b, 1:, :] = red[:NS].reshape(KMAX, C)
            counts[b] = red[NS:]
            po = IPC * NACC + i * KMAX
            # PE partial: psum [C, KMAX] drained to rows 0..C-1
            S[b, 1:, :] += o[:C, po : po + KMAX].T
            # count partials from mask-chunks 1..
            c2o = IPC * NACC + IPC * KMAX + i * (NCHUNK - 1) * K1
            counts[b] += (
                o[:, c2o : c2o + (NCHUNK - 1) * K1]
                .sum(axis=0)
                .reshape(NCHUNK - 1, K1)
                .sum(axis=0)
            )

    # scalar epilogue, mirroring reference.py
    N = np.linalg.norm(S, axis=-1)
    N[:, 0] = 0.0
    f = np.log(np.maximum(SIGMA_DIS - N, 0.0) ** 2 + 1.0)
    sum_g = (counts * f).sum(axis=-1)
    present = counts > 0
    Kb = np.where(
        present.any(axis=1), (present * np.arange(K1)).max(axis=1), 0
    ).astype(np.float64)
    active = Kb > 1.0
    Pn = Kb * (Kb - 1.0) * 0.5
    own = np.where(active, (Kb - 1.0) * sum_g + HW * (Pn - (Kb - 1.0)) * F0, 0.0)
    P_act = np.where(active, Pn, 0.0)
    other = (P_act.sum() - P_act) * HW * F0
    scale = np.where(active, 1.0 / (Kb * (Kb - 1.0)), Kb)
    return np.float32((scale * (own + other)).sum())



# revision 6
# speedup vs baseline: 1.5226x; 1.5226x over previous
"""Trainium2 Bass kernel for nn_DiscriminationLoss (segment_reduce).

Math: the reference loss reduces to, per image b:
  S[b,k,c]    = sum of pred[b,c] over pixels with label k   (k=1..16 needed)
  counts[b,k] = histogram of labels                          (k=0..16)
followed by a tiny scalar epilogue on the host:
  N = ||S||_2 over c, N[0]=0; f = log(relu(3-N)^2+1)
  sum_g = counts . f; own/other/scale pair-combination; final scalar sum.

Device strategy (per core, 2 images, data-parallel over batch):
- Pixels live as [128 partitions, 6400 columns]; a "chunk" is one column
  (128 pixels), a "group" is 8 consecutive chunks.
- DVE builds, per group, a [128, 128] bf16 mask slab whose column (k-1)*8+i
  is the indicator [label == k] for chunk 8g+i.  One tensor_scalar(is_equal)
  per k spans a whole multi-group slab with a 3D strided output AP, which
  keeps the DVE 4x performance mode (2-byte dtype, innermost-contiguous).
  accum_out on the same ops yields the per-partition histogram for free.
- PE consumes each group with ONE matmul: stationary = mask slab [128,128]
  (LdWeights), moving = pred fp8 [128, 32] (8 chunks x 4 channels), PSUM
  [128, 32] accumulated over all groups of an image.  Only the 8 diagonal
  (chunk_i == chunk_j) blocks are meaningful; the host sums
  psum[(k-1)*8+i, i*4+c] over i to get S[b,k,c].  Off-diagonal products are
  computed-but-never-read garbage.  This amortizes the moving stream to
  4 columns per chunk -> ~13us PE vs ~43us for per-chunk matmuls.
- pred is fp8e4m3 (host-converted): segment sums only need to clear the
  relu(3-||S||) threshold with ||S|| ~ 300, so 4% element error is noise;
  halves DMA bytes vs bf16.  Labels are bf16 (exact for 0..16; 2-byte dtype
  needed for the DVE 4x mode).

Toolchain workarounds (kept from the previous kernel):
- walrus rejects instructions carrying more than one sync wait; the BIR is
  post-processed to split multi-wait instructions into single-wait Drains.
- sem waits must not ride on tensor_scalar compute: a tiny DVE tensor_copy
  "absorber" takes each DMA-completion / buffer-reuse wait, and later DVE
  ops are same-engine ordered behind it.

Sharding: data-parallel over batch, 2 images per core, no collectives.
"""

import json

import numpy as np
import ml_dtypes

import concourse.bass as bass
import concourse.mybir as mybir
import concourse.tile as tile
import concourse.bass2jax as _b2j
from concourse.bass_utils import run_bass_kernel_spmd


def _split_multiwait_bir(bir_json: bytes) -> bytes:
    """walrus in this container rejects instructions carrying more than one
    sync wait. Tile's kernel-tail drain aggregates one wait per DMA/engine
    sem lane onto a single SP Drain, so split any multi-wait instruction
    into single-wait predecessors on the same engine."""
    d = json.loads(bir_json)
    changed = False
    for fn in d.get("functions", []):
        for bb in fn.get("blocks", []):
            insts = bb.get("instructions", [])
            out = []
            for ins in insts:
                si = ins.get("sync_info") or {}
                waits = si.get("on_wait") or []
                if len(waits) > 1:
                    changed = True
                    for wi, w in enumerate(waits[:-1]):
                        out.append(
                            {
                                "debug": ins.get("debug"),
                                "engine": ins["engine"],
                                "ins": [],
                                "is_reset_sema": False,
                                "name": f"{ins['name']}_w{wi}",
                                "opcode": "Drain",
                                "outs": [],
                                "sync_info": {"on_update": [], "on_wait": [w]},
                            }
                        )
                    si["on_wait"] = [waits[-1]]
                out.append(ins)
            bb["instructions"] = out
    if not changed:
        return bir_json
    return json.dumps(d).encode()


_ORIG_COMPILE_BIR = _b2j.compile_bir_kernel


def _compile_bir_splitting_waits(bir_json, tmpdir, neff_name="file.neff"):
    return _ORIG_COMPILE_BIR(_split_multiwait_bir(bir_json), tmpdir, neff_name=neff_name)


_b2j.compile_bir_kernel = _compile_bir_splitting_waits

B, C, H, W = 16, 4, 640, 640
HW = H * W                 # 409600
P = 128
FD = HW // P               # 3200 columns per image
N_CORES = 8
IPC = B // N_CORES         # images per core
KMAX = 16
K1 = KMAX + 1
SIGMA_DIS = 3.0
F0 = float(np.log(SIGMA_DIS**2 + 1.0))

NCOL = IPC * FD            # 6400 pixel-columns per core
NG = NCOL // 8             # 800 groups of 8 chunks
# slab sizes in groups; must not cross the image boundary (group 400).
# small first slab -> compute starts early; small last slab -> short tail.
SLABS = [100, 300, 300, 100]
assert sum(SLABS) == NG and sum(SLABS[:2]) == NG // 2
NSLAB = len(SLABS)

# out layout: [128, 64 cnt | 32 psum img0 | 32 psum img1] f32
CNT_COLS = NSLAB * KMAX    # 64: per-slab per-k partition histogram partials
OUT_COLS = CNT_COLS + IPC * 32

# test.py can set RUN_KWARGS["trace"] = True and read LAST_RESULT for profiling
RUN_KWARGS = {}
LAST_RESULT = None
_NC_CACHE = []

BF16 = mybir.dt.bfloat16
FP8 = mybir.dt.float8e4
F32 = mybir.dt.float32


def _build_nc():
    nc = bass.Bass("TRN2", target_bir_lowering=False, debug=False)
    pred_d = nc.dram_tensor("pred", [P, NCOL * C], FP8, kind="ExternalInput")
    lab_d = nc.dram_tensor("lab", [P, NCOL], BF16, kind="ExternalInput")
    out_d = nc.dram_tensor("out", [P, OUT_COLS], F32, kind="ExternalOutput")

    with tile.TileContext(nc) as tc:
        with tc.tile_pool(name="pool", bufs=1) as pool, \
             tc.tile_pool(name="ps", bufs=2, space="PSUM") as pspool:
            pred_sb = pool.tile([P, NCOL * C], FP8, name="pred_sb")
            lab_sb = pool.tile([P, NCOL], BF16, name="lab_sb")
            acc = pool.tile([P, OUT_COLS], F32, name="acc")
            dummy = pool.tile([P, 2], FP8, name="dummy")

            # per-slab input DMAs so compute starts as slices land
            g0 = 0
            for s, gs in enumerate(SLABS):
                lo, hi = g0 * 8, (g0 + gs) * 8
                nc.sync.dma_start(lab_sb[:, lo:hi], lab_d[:, lo:hi])
                nc.sync.dma_start(pred_sb[:, lo * C:hi * C], pred_d[:, lo * C:hi * C])
                g0 += gs

            pred4 = pred_sb[:].rearrange("p (g m) -> p g m", m=32)  # [P, NG, 32]

            psum = [pspool.tile([P, 32], F32, name=f"ps_{i}") for i in range(IPC)]
            g0 = 0
            for s, gs in enumerate(SLABS):
                img = (2 * g0) // NG
                slab = pool.tile([P, gs * 128], BF16, name=f"slab_{s}",
                                 tag="slab", bufs=2)
                slab3 = slab[:].rearrange("p (g m) -> p g m", m=128)
                lab3 = lab_sb[:, g0 * 8:(g0 + gs) * 8].rearrange(
                    "p (g i) -> p g i", i=8)
                # absorber: takes the lab-DMA wait (and slab-buffer WAR wait)
                # so the tensor_scalar ops below carry no sem waits
                nc.vector.tensor_copy(slab[:, 0:2], lab_sb[:, g0 * 8:g0 * 8 + 2])
                for k in range(1, K1):
                    nc.vector.tensor_scalar(
                        out=slab3[:, :, (k - 1) * 8:k * 8],
                        in0=lab3[:],
                        scalar1=float(k),
                        scalar2=None,
                        op0=mybir.AluOpType.is_equal,
                        op1=mybir.AluOpType.add,
                        accum_out=acc[:, s * KMAX + (k - 1):s * KMAX + k],
                    )
                # absorber for the pred-DMA wait on the PE side: the first
                # matmul of each slab would otherwise carry the DMA sem wait
                # alongside its slab-ready wait.
                nc.vector.tensor_copy(dummy[:], pred_sb[:, g0 * 32:g0 * 32 + 2])
                for gl in range(gs):
                    g = g0 + gl
                    nc.tensor.matmul(
                        psum[img][:],
                        slab3[:, gl, :],
                        pred4[:, g, :],
                        start=(g % (NG // 2) == 0),
                        stop=(g % (NG // 2) == NG // 2 - 1),
                    )
                g0 += gs

            for i in range(IPC):
                nc.vector.tensor_copy(
                    acc[:, CNT_COLS + i * 32:CNT_COLS + (i + 1) * 32], psum[i][:]
                )
            nc.gpsimd.dma_start(out_d[:], acc[:])
    return nc


def _get_nc():
    if not _NC_CACHE:
        _NC_CACHE.append(_build_nc())
    return _NC_CACHE[0]


def make_in_maps(pred_similarities, kernel_labels):
    pred = np.ascontiguousarray(pred_similarities, dtype=np.float32).reshape(
        N_CORES, IPC, C, P, FD
    )
    # fp8 e4m3 conversion; |pred| <= ~6 sigma so no saturation concerns
    pred8 = pred.astype(mybir.dt.np(FP8))
    # -> [cores, P, IPC, FD, C] so column t*4+c matches chunk-major layout
    pred8 = pred8.transpose(0, 3, 1, 4, 2).reshape(N_CORES, P, NCOL * C)

    labs = np.ascontiguousarray(kernel_labels, dtype=np.float32).reshape(
        N_CORES, IPC, P, FD
    )
    labs16 = labs.astype(ml_dtypes.bfloat16).transpose(0, 2, 1, 3).reshape(
        N_CORES, P, NCOL
    )
    return [
        {"pred": np.ascontiguousarray(pred8[i]), "lab": np.ascontiguousarray(labs16[i])}
        for i in range(N_CORES)
    ]


def kernel(pred_similarities, kernel_labels):
    global LAST_RESULT
    nc = _get_nc()
    in_maps = make_in_maps(pred_similarities, kernel_labels)
    res = run_bass_kernel_spmd(nc, in_maps, core_ids=list(range(N_CORES)), **RUN_KWARGS)
    LAST_RESULT = res
    outs = [np.asarray(res.results[c]["out"]) for c in range(N_CORES)]
    return epilogue(outs)


def epilogue(outs):
    S = np.zeros((B, K1, C), np.float64)
    counts = np.zeros((B, K1), np.float64)
    half = NSLAB // 2
    for core in range(N_CORES):
        o = outs[core].astype(np.float64)  # [P, OUT_COLS]
        for i in range(IPC):
            b = core * IPC + i
            # histogram: sum partition partials of this image's slabs
            cnt = o[:, :CNT_COLS].reshape(P, NSLAB, KMAX)
            counts[b, 1:] = cnt[:, i * half:(i + 1) * half, :].sum(axis=(0, 1))
            counts[b, 0] = HW - counts[b, 1:].sum()
            # S: sum the 8 diagonal chunk-slot blocks of the psum tile
            ps = o[:, CNT_COLS + i * 32:CNT_COLS + (i + 1) * 32]  # [128, 32]
            ps4 = ps.reshape(KMAX, 8, 8, C)  # [k-1, i_row, i_col, c]
            S[b, 1:, :] = np.einsum("kiic->kc", ps4)

    # scalar epilogue, mirroring reference.py
    N = np.linalg.norm(S, axis=-1)
    N[:, 0] = 0.0
    f = np.log(np.maximum(SIGMA_DIS - N, 0.0) ** 2 + 1.0)
    sum_g = (counts * f).sum(axis=-1)
    present = counts > 0
    Kb = np.where(
        present.any(axis=1), (present * np.arange(K1)).max(axis=1), 0
    ).astype(np.float64)
    active = Kb > 1.0
    Pn = Kb * (Kb - 1.0) * 0.5
    own = np.where(active, (Kb - 1.0) * sum_g + HW * (Pn - (Kb - 1.0)) * F0, 0.0)
    P_act = np.where(active, Pn, 0.0)
    other = (P_act.sum() - P_act) * HW * F0
    scale = np.where(active, 1.0 / (Kb * (Kb - 1.0)), Kb)
    return np.float32((scale * (own + other)).sum())


# revision 21
# speedup vs baseline: 1.6454x; 1.0806x over previous
"""Trainium2 Bass kernel for nn_DiscriminationLoss (segment_reduce).

Math: the reference loss reduces to, per image b:
  S[b,k,c]    = sum of pred[b,c] over pixels with label k   (k=1..16 needed)
  counts[b,k] = histogram of labels                          (k=0..16)
followed by a tiny scalar epilogue on the host:
  N = ||S||_2 over c, N[0]=0; f = log(relu(3-N)^2+1)
  sum_g = counts . f; own/other/scale pair-combination; final scalar sum.

Device strategy (per core, 2 images, data-parallel over batch):
- Pixels live as [128 partitions, 6400 columns]; a "chunk" is one column
  (128 pixels), a "group" is 8 consecutive chunks.
- DVE builds, per group, a [128, 128] bf16 mask slab whose column (k-1)*8+i
  is the indicator [label == k] for chunk 8g+i.  One tensor_scalar(is_equal)
  per k spans a whole multi-group slab with a 3D strided output AP, which
  keeps the DVE 4x performance mode (2-byte dtype, innermost-contiguous).
  accum_out on the same ops yields the per-partition histogram for free.
- PE consumes each group with ONE matmul: stationary = mask slab [128,128]
  (LdWeights), moving = pred fp8 [128, 32] (8 chunks x 4 channels), PSUM
  [128, 32] accumulated over all groups of an image.  Only the 8 diagonal
  (chunk_i == chunk_j) blocks are meaningful; the host sums
  psum[(k-1)*8+i, i*4+c] over i to get S[b,k,c].  Off-diagonal products are
  computed-but-never-read garbage.  This amortizes the moving stream to
  4 columns per chunk -> ~13us PE vs ~43us for per-chunk matmuls.
- pred is fp8e4m3 (host-converted): segment sums only need to clear the
  relu(3-||S||) threshold with ||S|| ~ 300, so 4% element error is noise;
  halves DMA bytes vs bf16.  Labels are bf16 (exact for 0..16; 2-byte dtype
  needed for the DVE 4x mode).

Toolchain workarounds (kept from the previous kernel):
- walrus rejects instructions carrying more than one sync wait; the BIR is
  post-processed to split multi-wait instructions into single-wait Drains.
- sem waits must not ride on tensor_scalar compute: a tiny DVE tensor_copy
  "absorber" takes each DMA-completion / buffer-reuse wait, and later DVE
  ops are same-engine ordered behind it.

Sharding: data-parallel over batch, 2 images per core, no collectives.
"""

import json

import numpy as np
import ml_dtypes

import concourse.bass as bass
import concourse.mybir as mybir
import concourse.tile as tile
import concourse.bass2jax as _b2j
from concourse.bass_utils import run_bass_kernel_spmd


def _split_multiwait_bir(bir_json: bytes) -> bytes:
    """walrus in this container rejects instructions carrying more than one
    sync wait. Tile's kernel-tail drain aggregates one wait per DMA/engine
    sem lane onto a single SP Drain, so split any multi-wait instruction
    into single-wait predecessors on the same engine."""
    d = json.loads(bir_json)
    changed = False
    for fn in d.get("functions", []):
        for bb in fn.get("blocks", []):
            insts = bb.get("instructions", [])
            out = []
            for ins in insts:
                si = ins.get("sync_info") or {}
                waits = si.get("on_wait") or []
                if len(waits) > 1:
                    changed = True
                    for wi, w in enumerate(waits[:-1]):
                        out.append(
                            {
                                "debug": ins.get("debug"),
                                "engine": ins["engine"],
                                "ins": [],
                                "is_reset_sema": False,
                                "name": f"{ins['name']}_w{wi}",
                                "opcode": "Drain",
                                "outs": [],
                                "sync_info": {"on_update": [], "on_wait": [w]},
                            }
                        )
                    si["on_wait"] = [waits[-1]]
                out.append(ins)
            bb["instructions"] = out
    if not changed:
        return bir_json
    return json.dumps(d).encode()


_ORIG_COMPILE_BIR = _b2j.compile_bir_kernel


def _compile_bir_splitting_waits(bir_json, tmpdir, neff_name="file.neff"):
    return _ORIG_COMPILE_BIR(_split_multiwait_bir(bir_json), tmpdir, neff_name=neff_name)


_b2j.compile_bir_kernel = _compile_bir_splitting_waits

B, C, H, W = 16, 4, 640, 640
HW = H * W                 # 409600
P = 128
FD = HW // P               # 3200 columns per image
N_CORES = 8
IPC = B // N_CORES         # images per core
KMAX = 16
K1 = KMAX + 1
SIGMA_DIS = 3.0
F0 = float(np.log(SIGMA_DIS**2 + 1.0))

NCOL = IPC * FD            # 6400 pixel-columns per core
NG = NCOL // 8             # 800 groups of 8 chunks
# slab sizes in groups; must not cross the image boundary (group 400).
# small first slab -> compute starts early; small last slab -> short tail.
SLABS = [100, 300, 300, 100]
assert sum(SLABS) == NG and sum(SLABS[:2]) == NG // 2
NSLAB = len(SLABS)

# out layout: [128, 64 cnt | 32 psum img0 | 32 psum img1] f32
CNT_COLS = NSLAB * KMAX    # 64: per-slab per-k partition histogram partials
OUT_COLS = CNT_COLS + IPC * 32

ACT_K = 14                 # mask planes built on the Activation engine
ACT_K2 = 13
# warm-up filler matmuls issued before each slab's real matmuls (tunable)
FILLERS = [0, 0, 0, 0]

# test.py can set RUN_KWARGS["trace"] = True and read LAST_RESULT for profiling
RUN_KWARGS = {}
LAST_RESULT = None
_NC_CACHE = []

BF16 = mybir.dt.bfloat16
FP8 = mybir.dt.float8e4
F32 = mybir.dt.float32
AF = mybir.ActivationFunctionType


def _build_nc():
    nc = bass.Bass("TRN2", target_bir_lowering=False, debug=False)
    pred_d = nc.dram_tensor("pred", [P, NCOL * C], FP8, kind="ExternalInput")
    lab_d = nc.dram_tensor("lab", [P, NCOL], BF16, kind="ExternalInput")
    out_d = nc.dram_tensor("out", [P, OUT_COLS], F32, kind="ExternalOutput")

    with tile.TileContext(nc) as tc:
        with tc.tile_pool(name="pool", bufs=1) as pool, \
             tc.tile_pool(name="ps", bufs=2, space="PSUM") as pspool:
            pred_sb = pool.tile([P, NCOL * C], FP8, name="pred_sb")
            lab_sb = pool.tile([P, NCOL], BF16, name="lab_sb")
            acc = pool.tile([P, OUT_COLS], F32, name="acc")
            dummy = pool.tile([P, 2], FP8, name="dummy")
            # per-partition scalars for Act-engine mask building
            consts = pool.tile([P, 5], F32, name="consts")
            nc.gpsimd.memset(consts[:, 1:2], 1.0)             # bias +1
            nc.gpsimd.memset(consts[:, 2:3], -1.0)            # scale -1
            nc.gpsimd.memset(consts[:, 3:4], float(-ACT_K))   # bias -k
            nc.gpsimd.memset(consts[:, 4:5], float(-ACT_K2))  # bias -k2
            sq = pool.tile([P, max(SLABS) * 8], BF16, name="sq")
            # PE warm-up fillers: keep the tensor engine busy through DVE
            # mask-build gaps so it runs ramped (2.4GHz) when real matmuls
            # arrive, instead of restarting at the 1.2GHz p-state each burst
            fstat = pool.tile([P, 8], BF16, name="fstat")
            fmov = pool.tile([P, 8], FP8, name="fmov")
            nc.gpsimd.memset(fstat[:], 0.0)
            nc.gpsimd.memset(fmov[:], 0.0)

            # per-slab input DMAs so compute starts as slices land; labels
            # are front-loaded (DVE consumes them first and is the critical
            # engine), pred interleaves behind
            bounds = []
            g0 = 0
            for gs in SLABS:
                bounds.append((g0, g0 + gs))
                g0 += gs
            nc.sync.dma_start(lab_sb[:, bounds[0][0] * 8:bounds[0][1] * 8],
                              lab_d[:, bounds[0][0] * 8:bounds[0][1] * 8])
            nc.sync.dma_start(lab_sb[:, bounds[1][0] * 8:bounds[1][1] * 8],
                              lab_d[:, bounds[1][0] * 8:bounds[1][1] * 8])
            nc.sync.dma_start(pred_sb[:, bounds[0][0] * 32:bounds[0][1] * 32],
                              pred_d[:, bounds[0][0] * 32:bounds[0][1] * 32])
            nc.sync.dma_start(lab_sb[:, bounds[2][0] * 8:bounds[3][1] * 8],
                              lab_d[:, bounds[2][0] * 8:bounds[3][1] * 8])
            for lo, hi in bounds[1:]:
                nc.sync.dma_start(pred_sb[:, lo * 32:hi * 32],
                                  pred_d[:, lo * 32:hi * 32])

            pred4 = pred_sb[:].rearrange("p (g m) -> p g m", m=32)  # [P, NG, 32]

            psum = [pspool.tile([P, 32], F32, name=f"ps_{i}") for i in range(IPC)]
            fgarb = pspool.tile([8, 8], F32, name="fgarb")
            g0 = 0
            for s, gs in enumerate(SLABS):
                img = (2 * g0) // NG
                slab = pool.tile([P, gs * 128], BF16, name=f"slab_{s}",
                                 tag="slab", bufs=2)
                slab3 = slab[:].rearrange("p (g m) -> p g m", m=128)
                lab3 = lab_sb[:, g0 * 8:(g0 + gs) * 8].rearrange(
                    "p (g i) -> p g i", i=8)
                # absorbers: take the lab-DMA wait (and slab-buffer WAR wait)
                # so the tensor_scalar ops below carry no sem waits
                nc.vector.tensor_copy(slab[:, 0:2], lab_sb[:, g0 * 8:g0 * 8 + 2])
                nc.gpsimd.tensor_copy(slab[:, 2:4], lab_sb[:, g0 * 8:g0 * 8 + 2])
                act_ks = (ACT_K, ACT_K2)
                for k in range(1, K1):
                    # walrus only allows tensor_scalar on DVE (the Pool
                    # engine fails its codegen check), so the split is
                    # 14 planes DVE + 2 planes Act
                    if k in act_ks:
                        continue
                    eng = nc.vector
                    eng.tensor_scalar(
                        out=slab3[:, :, (k - 1) * 8:k * 8],
                        in0=lab3[:],
                        scalar1=float(k),
                        scalar2=None,
                        op0=mybir.AluOpType.is_equal,
                        op1=mybir.AluOpType.add,
                        accum_out=acc[:, s * KMAX + (k - 1):s * KMAX + k],
                    )
                # Act-engine planes: [lab==k] == relu(1 - (lab-k)^2) for
                # integer labels; accum_out gives the histogram column
                for ki, k in enumerate(act_ks):
                    bias = consts[:, 3 + ki:4 + ki]
                    nc.scalar.activation(sq[:, :gs * 8],
                                         lab3[:].rearrange("p g i -> p (g i)"),
                                         AF.Square, bias=bias)
                    nc.scalar.activation(
                        slab3[:, :, (k - 1) * 8:k * 8],
                        sq[:, :gs * 8].rearrange("p (g i) -> p g i", i=8),
                        AF.Relu, bias=consts[:, 1:2], scale=consts[:, 2:3],
                        accum_out=acc[:, s * KMAX + (k - 1):s * KMAX + k],
                    )
                # absorber for the pred-DMA wait on the PE side: the first
                # matmul of each slab would otherwise carry the DMA sem wait
                # alongside its slab-ready wait.
                nc.vector.tensor_copy(dummy[:], pred_sb[:, g0 * 32:g0 * 32 + 2])
                for _ in range(FILLERS[s]):
                    nc.tensor.matmul(fgarb[:], fstat[:], fmov[:],
                                     start=True, stop=True)
                for gl in range(gs):
                    g = g0 + gl
                    nc.tensor.matmul(
                        psum[img][:],
                        slab3[:, gl, :],
                        pred4[:, g, :],
                        start=(g % (NG // 2) == 0),
                        stop=(g % (NG // 2) == NG // 2 - 1),
                    )
                g0 += gs

            for i in range(IPC):
                nc.vector.tensor_copy(
                    acc[:, CNT_COLS + i * 32:CNT_COLS + (i + 1) * 32], psum[i][:]
                )
            nc.gpsimd.dma_start(out_d[:], acc[:])
    return nc


def _get_nc():
    if not _NC_CACHE:
        _NC_CACHE.append(_build_nc())
    return _NC_CACHE[0]


def make_in_maps(pred_similarities, kernel_labels):
    pred = np.ascontiguousarray(pred_similarities, dtype=np.float32).reshape(
        N_CORES, IPC, C, P, FD
    )
    # fp8 e4m3 conversion; |pred| <= ~6 sigma so no saturation concerns
    pred8 = pred.astype(mybir.dt.np(FP8))
    # -> [cores, P, IPC, FD, C] so column t*4+c matches chunk-major layout
    pred8 = pred8.transpose(0, 3, 1, 4, 2).reshape(N_CORES, P, NCOL * C)

    labs = np.ascontiguousarray(kernel_labels, dtype=np.float32).reshape(
        N_CORES, IPC, P, FD
    )
    labs16 = labs.astype(ml_dtypes.bfloat16).transpose(0, 2, 1, 3).reshape(
        N_CORES, P, NCOL
    )
    return [
        {"pred": np.ascontiguousarray(pred8[i]), "lab": np.ascontiguousarray(labs16[i])}
        for i in range(N_CORES)
    ]


def kernel(pred_similarities, kernel_labels):
    global LAST_RESULT
    nc = _get_nc()
    in_maps = make_in_maps(pred_similarities, kernel_labels)
    res = run_bass_kernel_spmd(nc, in_maps, core_ids=list(range(N_CORES)), **RUN_KWARGS)
    LAST_RESULT = res
    outs = [np.asarray(res.results[c]["out"]) for c in range(N_CORES)]
    return epilogue(outs)


def epilogue(outs):
    S = np.zeros((B, K1, C), np.float64)
    counts = np.zeros((B, K1), np.float64)
    half = NSLAB // 2
    for core in range(N_CORES):
        o = outs[core].astype(np.float64)  # [P, OUT_COLS]
        for i in range(IPC):
            b = core * IPC + i
            # histogram: sum partition partials of this image's slabs
            cnt = o[:, :CNT_COLS].reshape(P, NSLAB, KMAX)
            counts[b, 1:] = cnt[:, i * half:(i + 1) * half, :].sum(axis=(0, 1))
            counts[b, 0] = HW - counts[b, 1:].sum()
            # S: sum the 8 diagonal chunk-slot blocks of the psum tile
            ps = o[:, CNT_COLS + i * 32:CNT_COLS + (i + 1) * 32]  # [128, 32]
            ps4 = ps.reshape(KMAX, 8, 8, C)  # [k-1, i_row, i_col, c]
            S[b, 1:, :] = np.einsum("kiic->kc", ps4)

    # scalar epilogue, mirroring reference.py
    N = np.linalg.norm(S, axis=-1)
    N[:, 0] = 0.0
    f = np.log(np.maximum(SIGMA_DIS - N, 0.0) ** 2 + 1.0)
    sum_g = (counts * f).sum(axis=-1)
    present = counts > 0
    Kb = np.where(
        present.any(axis=1), (present * np.arange(K1)).max(axis=1), 0
    ).astype(np.float64)
    active = Kb > 1.0
    Pn = Kb * (Kb - 1.0) * 0.5
    own = np.where(active, (Kb - 1.0) * sum_g + HW * (Pn - (Kb - 1.0)) * F0, 0.0)
    P_act = np.where(active, Pn, 0.0)
    other = (P_act.sum() - P_act) * HW * F0
    scale = np.where(active, 1.0 / (Kb * (Kb - 1.0)), Kb)
    return np.float32((scale * (own + other)).sum())


# revision 28
# speedup vs baseline: 1.7216x; 1.0463x over previous
"""Trainium2 Bass kernel for nn_DiscriminationLoss (segment_reduce).

Math: the reference loss reduces to, per image b:
  S[b,k,c]    = sum of pred[b,c] over pixels with label k   (k=1..16 needed)
  counts[b,k] = histogram of labels                          (k=0..16)
followed by a tiny scalar epilogue on the host:
  N = ||S||_2 over c, N[0]=0; f = log(relu(3-N)^2+1)
  sum_g = counts . f; own/other/scale pair-combination; final scalar sum.

Device strategy (per core, 2 images, data-parallel over batch):
- Pixels live as [128 partitions, 6400 columns]; a "chunk" is one column
  (128 pixels), a "group" is 8 consecutive chunks.
- DVE builds, per group, a [128, 128] bf16 mask slab whose column (k-1)*8+i
  is the indicator [label == k] for chunk 8g+i.  One tensor_scalar(is_equal)
  per k spans a whole multi-group slab with a 3D strided output AP, which
  keeps the DVE 4x performance mode (2-byte dtype, innermost-contiguous).
  accum_out on the same ops yields the per-partition histogram for free.
- PE consumes each group with ONE matmul: stationary = mask slab [128,128]
  (LdWeights), moving = pred fp8 [128, 32] (8 chunks x 4 channels), PSUM
  [128, 32] accumulated over all groups of an image.  Only the 8 diagonal
  (chunk_i == chunk_j) blocks are meaningful; the host sums
  psum[(k-1)*8+i, i*4+c] over i to get S[b,k,c].  Off-diagonal products are
  computed-but-never-read garbage.  This amortizes the moving stream to
  4 columns per chunk -> ~13us PE vs ~43us for per-chunk matmuls.
- pred is fp8e4m3 (host-converted): segment sums only need to clear the
  relu(3-||S||) threshold with ||S|| ~ 300, so 4% element error is noise;
  halves DMA bytes vs bf16.  Labels are bf16 (exact for 0..16; 2-byte dtype
  needed for the DVE 4x mode).

Toolchain workarounds (kept from the previous kernel):
- walrus rejects instructions carrying more than one sync wait; the BIR is
  post-processed to split multi-wait instructions into single-wait Drains.
- sem waits must not ride on tensor_scalar compute: a tiny DVE tensor_copy
  "absorber" takes each DMA-completion / buffer-reuse wait, and later DVE
  ops are same-engine ordered behind it.

Sharding: data-parallel over batch, 2 images per core, no collectives.
"""

import json

import numpy as np
import ml_dtypes

import concourse.bass as bass
import concourse.mybir as mybir
import concourse.tile as tile
import concourse.bass2jax as _b2j
from concourse.bass_utils import run_bass_kernel_spmd


def _split_multiwait_bir(bir_json: bytes) -> bytes:
    """walrus in this container rejects instructions carrying more than one
    sync wait. Tile's kernel-tail drain aggregates one wait per DMA/engine
    sem lane onto a single SP Drain, so split any multi-wait instruction
    into single-wait predecessors on the same engine."""
    d = json.loads(bir_json)
    changed = False
    for fn in d.get("functions", []):
        for bb in fn.get("blocks", []):
            insts = bb.get("instructions", [])
            out = []
            for ins in insts:
                si = ins.get("sync_info") or {}
                waits = si.get("on_wait") or []
                if len(waits) > 1:
                    changed = True
                    for wi, w in enumerate(waits[:-1]):
                        out.append(
                            {
                                "debug": ins.get("debug"),
                                "engine": ins["engine"],
                                "ins": [],
                                "is_reset_sema": False,
                                "name": f"{ins['name']}_w{wi}",
                                "opcode": "Drain",
                                "outs": [],
                                "sync_info": {"on_update": [], "on_wait": [w]},
                            }
                        )
                    si["on_wait"] = [waits[-1]]
                out.append(ins)
            bb["instructions"] = out
    if not changed:
        return bir_json
    return json.dumps(d).encode()


_ORIG_COMPILE_BIR = _b2j.compile_bir_kernel


def _compile_bir_splitting_waits(bir_json, tmpdir, neff_name="file.neff"):
    return _ORIG_COMPILE_BIR(_split_multiwait_bir(bir_json), tmpdir, neff_name=neff_name)


_b2j.compile_bir_kernel = _compile_bir_splitting_waits

B, C, H, W = 16, 4, 640, 640
HW = H * W                 # 409600
P = 128
FD = HW // P               # 3200 columns per image
N_CORES = 8
IPC = B // N_CORES         # images per core
KMAX = 16
K1 = KMAX + 1
SIGMA_DIS = 3.0
F0 = float(np.log(SIGMA_DIS**2 + 1.0))

NCOL = IPC * FD            # 6400 pixel-columns per core
NG = NCOL // 8             # 800 groups of 8 chunks
# slab sizes in groups; must not cross the image boundary (group 400).
# small first slab -> compute starts early; small last slab -> short tail.
SLABS = [80, 320, 290, 110]
assert sum(SLABS) == NG and sum(SLABS[:2]) == NG // 2
NSLAB = len(SLABS)

# out layout: [128, 64 cnt | 32 psum img0 | 32 psum img1a | 32 psum img1b]
# f32.  Image 1's accumulation closes in two banks (slab 2 / slab 3) so the
# big drain and the main out-DMA descriptor generation overlap the PE tail.
CNT_COLS = NSLAB * KMAX    # 64: per-slab per-k partition histogram partials
NPS = 3
OUT_COLS = CNT_COLS + NPS * 32

ACT_K = 14                 # mask planes built on the Activation engine
ACT_K2 = 13
# warm-up filler matmuls issued before each slab's real matmuls (tunable)
FILLERS = [0, 0, 0, 0]

# test.py can set RUN_KWARGS["trace"] = True and read LAST_RESULT for profiling
RUN_KWARGS = {}
LAST_RESULT = None
_NC_CACHE = []

BF16 = mybir.dt.bfloat16
FP8 = mybir.dt.float8e4
F32 = mybir.dt.float32
AF = mybir.ActivationFunctionType


def _build_nc():
    nc = bass.Bass("TRN2", target_bir_lowering=False, debug=False)
    pred_d = nc.dram_tensor("pred", [P, NCOL * C], FP8, kind="ExternalInput")
    lab_d = nc.dram_tensor("lab", [P, NCOL], BF16, kind="ExternalInput")
    out_d = nc.dram_tensor("out", [P, OUT_COLS], F32, kind="ExternalOutput")

    with tile.TileContext(nc) as tc:
        with tc.tile_pool(name="pool", bufs=1) as pool, \
             tc.tile_pool(name="ps", bufs=2, space="PSUM") as pspool:
            pred_sb = pool.tile([P, NCOL * C], FP8, name="pred_sb")
            lab_sb = pool.tile([P, NCOL], BF16, name="lab_sb")
            acc = pool.tile([P, OUT_COLS], F32, name="acc")
            dummy = pool.tile([P, 2], FP8, name="dummy")
            # per-partition scalars for Act-engine mask building
            consts = pool.tile([P, 5], F32, name="consts")
            nc.gpsimd.memset(consts[:, 1:2], 1.0)             # bias +1
            nc.gpsimd.memset(consts[:, 2:3], -1.0)            # scale -1
            nc.gpsimd.memset(consts[:, 3:4], float(-ACT_K))   # bias -k
            nc.gpsimd.memset(consts[:, 4:5], float(-ACT_K2))  # bias -k2
            sq = pool.tile([P, max(SLABS) * 8], BF16, name="sq")
            # PE warm-up fillers: keep the tensor engine busy through DVE
            # mask-build gaps so it runs ramped (2.4GHz) when real matmuls
            # arrive, instead of restarting at the 1.2GHz p-state each burst
            fstat = pool.tile([P, 8], BF16, name="fstat")
            fmov = pool.tile([P, 8], FP8, name="fmov")
            nc.gpsimd.memset(fstat[:], 0.0)
            nc.gpsimd.memset(fmov[:], 0.0)

            # per-slab input DMAs so compute starts as slices land; labels
            # are front-loaded (DVE consumes them first and is the critical
            # engine), pred interleaves behind
            bounds = []
            g0 = 0
            for gs in SLABS:
                bounds.append((g0, g0 + gs))
                g0 += gs
            nc.sync.dma_start(lab_sb[:, bounds[0][0] * 8:bounds[0][1] * 8],
                              lab_d[:, bounds[0][0] * 8:bounds[0][1] * 8])
            nc.sync.dma_start(lab_sb[:, bounds[1][0] * 8:bounds[1][1] * 8],
                              lab_d[:, bounds[1][0] * 8:bounds[1][1] * 8])
            nc.sync.dma_start(pred_sb[:, bounds[0][0] * 32:bounds[0][1] * 32],
                              pred_d[:, bounds[0][0] * 32:bounds[0][1] * 32])
            nc.sync.dma_start(lab_sb[:, bounds[2][0] * 8:bounds[3][1] * 8],
                              lab_d[:, bounds[2][0] * 8:bounds[3][1] * 8])
            for lo, hi in bounds[1:]:
                nc.sync.dma_start(pred_sb[:, lo * 32:hi * 32],
                                  pred_d[:, lo * 32:hi * 32])

            pred4 = pred_sb[:].rearrange("p (g m) -> p g m", m=32)  # [P, NG, 32]

            psum = [pspool.tile([P, 32], F32, name=f"ps_{i}") for i in range(NPS)]
            fgarb = pspool.tile([8, 8], F32, name="fgarb")
            # accumulation segment boundaries (inclusive start, exclusive end)
            # and which psum tile each segment uses
            segs = [(0, NG // 2, 0), (NG // 2, NG - SLABS[-1], 1),
                    (NG - SLABS[-1], NG, 2)]

            def seg_of(g):
                for lo, hi, pi in segs:
                    if lo <= g < hi:
                        return lo, hi, pi
                raise AssertionError(g)
            g0 = 0
            for s, gs in enumerate(SLABS):
                img = (2 * g0) // NG
                slab = pool.tile([P, gs * 128], BF16, name=f"slab_{s}",
                                 tag="slab", bufs=2)
                slab3 = slab[:].rearrange("p (g m) -> p g m", m=128)
                lab3 = lab_sb[:, g0 * 8:(g0 + gs) * 8].rearrange(
                    "p (g i) -> p g i", i=8)
                # absorbers: take the lab-DMA wait (and slab-buffer WAR wait)
                # so the tensor_scalar ops below carry no sem waits
                nc.vector.tensor_copy(slab[:, 0:2], lab_sb[:, g0 * 8:g0 * 8 + 2])
                nc.gpsimd.tensor_copy(slab[:, 2:4], lab_sb[:, g0 * 8:g0 * 8 + 2])
                act_ks = (ACT_K, ACT_K2)
                for k in range(1, K1):
                    # walrus only allows tensor_scalar on DVE (the Pool
                    # engine fails its codegen check), so the split is
                    # 14 planes DVE + 2 planes Act
                    if k in act_ks:
                        continue
                    eng = nc.vector
                    eng.tensor_scalar(
                        out=slab3[:, :, (k - 1) * 8:k * 8],
                        in0=lab3[:],
                        scalar1=float(k),
                        scalar2=None,
                        op0=mybir.AluOpType.is_equal,
                        op1=mybir.AluOpType.add,
                        accum_out=acc[:, s * KMAX + (k - 1):s * KMAX + k],
                    )
                # Act-engine planes: [lab==k] == relu(1 - (lab-k)^2) for
                # integer labels; accum_out gives the histogram column
                for ki, k in enumerate(act_ks):
                    bias = consts[:, 3 + ki:4 + ki]
                    nc.scalar.activation(sq[:, :gs * 8],
                                         lab3[:].rearrange("p g i -> p (g i)"),
                                         AF.Square, bias=bias)
                    nc.scalar.activation(
                        slab3[:, :, (k - 1) * 8:k * 8],
                        sq[:, :gs * 8].rearrange("p (g i) -> p g i", i=8),
                        AF.Relu, bias=consts[:, 1:2], scale=consts[:, 2:3],
                        accum_out=acc[:, s * KMAX + (k - 1):s * KMAX + k],
                    )
                # absorber for the pred-DMA wait on the PE side: the first
                # matmul of each slab would otherwise carry the DMA sem wait
                # alongside its slab-ready wait.
                nc.vector.tensor_copy(dummy[:], pred_sb[:, g0 * 32:g0 * 32 + 2])
                for _ in range(FILLERS[s]):
                    nc.tensor.matmul(fgarb[:], fstat[:], fmov[:],
                                     start=True, stop=True)
                for gl in range(gs):
                    g = g0 + gl
                    lo, hi, pi = seg_of(g)
                    nc.tensor.matmul(
                        psum[pi][:],
                        slab3[:, gl, :],
                        pred4[:, g, :],
                        start=(g == lo),
                        stop=(g == hi - 1),
                    )
                g0 += gs

            for i in range(NPS):
                nc.vector.tensor_copy(
                    acc[:, CNT_COLS + i * 32:CNT_COLS + (i + 1) * 32], psum[i][:]
                )
            # main out-DMA (cnt + ps0 + ps1a) overlaps the last slab's
            # matmuls; only the tiny ps1b block ships after the PE finishes
            nc.gpsimd.dma_start(out_d[:, :OUT_COLS - 32], acc[:, :OUT_COLS - 32])
            nc.gpsimd.dma_start(out_d[:, OUT_COLS - 32:], acc[:, OUT_COLS - 32:])
    return nc


def _get_nc():
    if not _NC_CACHE:
        _NC_CACHE.append(_build_nc())
    return _NC_CACHE[0]


def make_in_maps(pred_similarities, kernel_labels):
    pred = np.ascontiguousarray(pred_similarities, dtype=np.float32).reshape(
        N_CORES, IPC, C, P, FD
    )
    # fp8 e4m3 conversion; |pred| <= ~6 sigma so no saturation concerns
    pred8 = pred.astype(mybir.dt.np(FP8))
    # -> [cores, P, IPC, FD, C] so column t*4+c matches chunk-major layout
    pred8 = pred8.transpose(0, 3, 1, 4, 2).reshape(N_CORES, P, NCOL * C)

    labs = np.ascontiguousarray(kernel_labels, dtype=np.float32).reshape(
        N_CORES, IPC, P, FD
    )
    labs16 = labs.astype(ml_dtypes.bfloat16).transpose(0, 2, 1, 3).reshape(
        N_CORES, P, NCOL
    )
    return [
        {"pred": np.ascontiguousarray(pred8[i]), "lab": np.ascontiguousarray(labs16[i])}
        for i in range(N_CORES)
    ]


def kernel(pred_similarities, kernel_labels):
    global LAST_RESULT
    nc = _get_nc()
    in_maps = make_in_maps(pred_similarities, kernel_labels)
    res = run_bass_kernel_spmd(nc, in_maps, core_ids=list(range(N_CORES)), **RUN_KWARGS)
    LAST_RESULT = res
    outs = [np.asarray(res.results[c]["out"]) for c in range(N_CORES)]
    return epilogue(outs)


def epilogue(outs):
    S = np.zeros((B, K1, C), np.float64)
    counts = np.zeros((B, K1), np.float64)
    half = NSLAB // 2
    for core in range(N_CORES):
        o = outs[core].astype(np.float64)  # [P, OUT_COLS]
        for i in range(IPC):
            b = core * IPC + i
            # histogram: sum partition partials of this image's slabs
            cnt = o[:, :CNT_COLS].reshape(P, NSLAB, KMAX)
            counts[b, 1:] = cnt[:, i * half:(i + 1) * half, :].sum(axis=(0, 1))
            counts[b, 0] = HW - counts[b, 1:].sum()
            # S: sum the 8 diagonal chunk-slot blocks of the psum block(s);
            # image 1's accumulation is split across two psum banks
            blocks = [1, 2] if i == 1 else [0]
            for pi in blocks:
                ps = o[:, CNT_COLS + pi * 32:CNT_COLS + (pi + 1) * 32]
                ps4 = ps.reshape(KMAX, 8, 8, C)  # [k-1, i_row, i_col, c]
                S[b, 1:, :] += np.einsum("kiic->kc", ps4)

    # scalar epilogue, mirroring reference.py
    N = np.linalg.norm(S, axis=-1)
    N[:, 0] = 0.0
    f = np.log(np.maximum(SIGMA_DIS - N, 0.0) ** 2 + 1.0)
    sum_g = (counts * f).sum(axis=-1)
    present = counts > 0
    Kb = np.where(
        present.any(axis=1), (present * np.arange(K1)).max(axis=1), 0
    ).astype(np.float64)
    active = Kb > 1.0
    Pn = Kb * (Kb - 1.0) * 0.5
    own = np.where(active, (Kb - 1.0) * sum_g + HW * (Pn - (Kb - 1.0)) * F0, 0.0)
    P_act = np.where(active, Pn, 0.0)
    other = (P_act.sum() - P_act) * HW * F0
    scale = np.where(active, 1.0 / (Kb * (Kb - 1.0)), Kb)
    return np.float32((scale * (own + other)).sum())
